# revision 1
# baseline (speedup 1.0000x reference)
"""Bass/Tile TRN2 kernel: 16-head MHA (B=2, T=2048, D=1024, H=64) on 8 NeuronCores.

Sharding: 8-way tensor parallel over heads — core c handles heads {2c, 2c+1}
for BOTH batches. After attention, one 8-core AllToAll swaps head-shards for
(batch, seq-block) shards, so each core runs the full output projection for
one 512-row output slice and no inter-core reduction is needed.

Per-core device pipeline (all FLOPs on device):
  - QKV projections as bf16 matmuls accumulating fp32 in PSUM. Activations
    arrive pre-transposed ([D, T]) so the contraction dim d sits on SBUF
    partitions; weight slices are per-core head slices.
  - Scores S^T[tk, tq] = K^T.T @ Q^T per head; the two heads are issued
    back-to-back as row-tiled (K=64, partitions 0-63 / 64-127) matmuls so they
    run concurrently on the PE array. 1/sqrt(H) is folded into Wq/bq on host.
  - exp on ScalarE straight out of PSUM (3-bank [128,1536] tiles), bf16 out.
  - PV matmul with a ones-augmented V (65 stationary columns) so row 64 of the
    PV accumulator is the softmax denominator for free.
  - Normalize on DVE (+ GPSIMD partition-broadcast of the reciprocal row).
  - AllToAll over all 8 cores: shard s = (batch s//4, tq-block s%4).
  - Output projection (full 1024-wide head contraction) for this core's
    512-row slice -> final output rows.
Host does layout-only prep (transpose, bf16 cast, weight slicing/replication)
and concatenates the 8 output slices.
"""

import os
import sys
from contextlib import ExitStack

import numpy as np

sys.path.insert(0, "/opt/trn_rl_repo")

import ml_dtypes  # noqa: E402

F16 = np.float16

B, T, D = 2, 2048, 1024
N_HEADS, H = 16, 64
NCORES = 8
GROUPS = [[0, 1, 2, 3, 4, 5, 6, 7]]
NLOC = 2            # heads per core
TQB = 512           # tq block size
NTQB = T // TQB     # 4
TKC = 128           # tk chunk size
NTKC = T // TKC     # 16
DC = 128            # d chunk
NDC = D // DC       # 8
SLOTS = 2           # score-psum slots per exp tile ([128, 2*512] = 2 banks)
OUT_ROWS = TQB      # 512 output rows per core
VA = 128            # V_aug stationary width: [V(64) | ones(1) | junk(63)]
NW = NLOC * H       # 128: projection output width per core

_CACHE = {}


def _legalize_waits(bir_bytes):
    """This toolchain's walrus accepts at most ONE semaphore wait per
    instruction ("Too many sync wait commands"). Tile's sem assignment emits
    several. Hoist all but one wait of each instruction onto same-engine NoOps
    inserted immediately before it (engines execute their stream in order, so
    waiting earlier on the same engine is equivalent)."""
    import json

    j = json.loads(bir_bytes)
    ctr = 0
    for fn in j["functions"]:
        for blk in fn["blocks"]:
            out = []
            for ins in blk["instructions"]:
                si = ins.get("sync_info")
                waits = (si or {}).get("on_wait") or []
                if len(waits) > 1:
                    for w in waits[:-1]:
                        ctr += 1
                        out.append(
                            {
                                "engine": ins["engine"],
                                "ins": [],
                                "outs": [],
                                "name": f"waitfix-{ctr}",
                                "opcode": "NoOp",
                                "sync_info": {"on_wait": [w], "on_update": []},
                            }
                        )
                    si["on_wait"] = [waits[-1]]
                out.append(ins)
            blk["instructions"] = out
    return json.dumps(j).encode()


def _build():
    import concourse.bass as bass
    import concourse.mybir as mybir
    import concourse.tile as tile

    f32 = mybir.dt.float32
    f16 = mybir.dt.float16
    bf16 = mybir.dt.bfloat16
    AF = mybir.ActivationFunctionType
    ALU = mybir.AluOpType

    nc = bass.Bass(
        "TRN2", target_bir_lowering=False, debug=False, num_devices=NCORES
    )

    qT = [nc.dram_tensor(f"qT{b}", [D, T], f16, kind="ExternalInput") for b in range(B)]
    kT = [nc.dram_tensor(f"kT{b}", [D, T], f16, kind="ExternalInput") for b in range(B)]
    vT = [nc.dram_tensor(f"vT{b}", [D, T], f16, kind="ExternalInput") for b in range(B)]
    wq = nc.dram_tensor("wq", [D, NW], f16, kind="ExternalInput")
    wk = nc.dram_tensor("wk", [D, NW], f16, kind="ExternalInput")
    wv = nc.dram_tensor("wv", [D, NW], f16, kind="ExternalInput")
    wp = nc.dram_tensor("wp", [N_HEADS * H, D], f16, kind="ExternalInput")
    bq = nc.dram_tensor("bq", [128, 1], f32, kind="ExternalInput")
    bk = nc.dram_tensor("bk", [128, 1], f32, kind="ExternalInput")
    bv = nc.dram_tensor("bv", [128, 1], f32, kind="ExternalInput")
    bp = nc.dram_tensor("bp", [128, D], f32, kind="ExternalInput")
    ident = nc.dram_tensor("ident", [128, 128], f16, kind="ExternalInput")
    out = nc.dram_tensor("out", [OUT_ROWS, D], f32, kind="ExternalOutput")

    with tile.TileContext(nc) as tc, ExitStack() as ctx:
        p_const = ctx.enter_context(tc.tile_pool(name="const", bufs=1))
        p_xt = ctx.enter_context(tc.tile_pool(name="xt", bufs=1))
        p_qk = ctx.enter_context(tc.tile_pool(name="qk", bufs=4))
        p_va = ctx.enter_context(tc.tile_pool(name="va", bufs=2))
        p_pt = ctx.enter_context(tc.tile_pool(name="pt", bufs=3))
        p_a = ctx.enter_context(tc.tile_pool(name="a", bufs=3))
        p_at = ctx.enter_context(tc.tile_pool(name="at", bufs=8))
        p_o = ctx.enter_context(tc.tile_pool(name="o", bufs=2))
        p_dram = ctx.enter_context(tc.tile_pool(name="dram", bufs=1, space="DRAM"))

        # ---- constant loads -------------------------------------------------
        wq_sb = p_const.tile([128, NDC * NW], f16)
        wk_sb = p_const.tile([128, NDC * NW], f16)
        wv_sb = p_const.tile([128, NDC * NW], f16)
        wp_sb = p_const.tile([128, (N_HEADS * H // 128) * D], f16)
        bq_sb = p_const.tile([128, 1], f32)
        bk_sb = p_const.tile([128, 1], f32)
        bv_sb = p_const.tile([128, 1], f32)
        bp_sb = p_const.tile([128, D], f32)
        id_sb = p_const.tile([128, 128], f16)
        nc.sync.dma_start(id_sb[:], ident[:])
        for sb_t, ext, m in (
            (wq_sb, wq, NW),
            (wk_sb, wk, NW),
            (wv_sb, wv, NW),
            (wp_sb, wp, D),
        ):
            nc.sync.dma_start(
                sb_t[:].rearrange("p (c m) -> p c m", m=m),
                ext[:].rearrange("(c p) m -> p c m", p=128),
            )
        for sb_t, ext in ((bq_sb, bq), (bk_sb, bk), (bv_sb, bv), (bp_sb, bp)):
            nc.sync.dma_start(sb_t[:], ext[:])

        a2a_in = p_dram.tile([NCORES * NW, TQB], f16)
        a2a_out = p_dram.tile([NCORES * NW, TQB], f16)

        ps_sc = ctx.enter_context(tc.tile_pool(name="ps_sc", bufs=3, space="PSUM"))
        ps_pv = ctx.enter_context(tc.tile_pool(name="ps_pv", bufs=2, space="PSUM"))

        def projections(b):
            if True:
                # activation loads (pre-transposed on host)
                xq = p_xt.tile([128, NDC * T], f16, name=f"xq{b}", tag="xq")
                xk = p_xt.tile([128, NDC * T], f16, name=f"xk{b}", tag="xk")
                xv = p_xt.tile([128, NDC * T], f16, name=f"xv{b}", tag="xv")
                for sb_t, ext in ((xv, vT[b]), (xk, kT[b]), (xq, qT[b])):
                    nc.sync.dma_start(
                        sb_t[:].rearrange("p (c t) -> p c t", t=T),
                        ext[:].rearrange("(c p) t -> p c t", p=128),
                    )

                # V projection as V^T [nh, t] (N=512 matmuls), then PE
                # transposes into V_aug [tk, (i, head, 128)] with ones columns
                vt = p_va.tile([128, T], f16, name=f"vt{b}", tag="vt")
                for tb in range(NTQB):
                    psv = ps_sc.tile([128, TQB], f32, name=f"psv{b}{tb}", tag="sc")
                    for dc in range(NDC):
                        nc.tensor.matmul(
                            psv[:],
                            lhsT=wv_sb[:, dc * NW : (dc + 1) * NW],
                            rhs=xv[:, dc * T + tb * TQB : dc * T + (tb + 1) * TQB],
                            start=(dc == 0),
                            stop=(dc == NDC - 1),
                        )
                    nc.vector.tensor_scalar(
                        vt[:, tb * TQB : (tb + 1) * TQB],
                        psv[:],
                        bv_sb[:, 0:1],
                        None,
                        ALU.add,
                    )
                va = p_va.tile(
                    [128, NTKC * NLOC * VA], f16, name=f"va{b}", tag="va"
                )
                nc.vector.memset(
                    va[:].rearrange("p (i h x) -> p i h x", h=NLOC, x=VA)[
                        :, :, :, H : H + 1
                    ],
                    1.0,
                )
                for i in range(NTKC):
                    pst = ps_sc.tile([128, 128], f16, name=f"pst{b}{i}", tag="sc")
                    nc.tensor.transpose(
                        pst[:], vt[:, i * TKC : (i + 1) * TKC], id_sb[:]
                    )
                    dst = va[:, i * NLOC * VA : (i + 1) * NLOC * VA].rearrange(
                        "p (h x) -> p h x", x=VA
                    )[:, :, 0:H]
                    nc.vector.tensor_copy(
                        dst, pst[:].rearrange("p (h x) -> p h x", x=H)
                    )

                # Q^T / K^T projections
                qt = p_qk.tile([128, T], f16, name=f"qt{b}", tag="qk")
                kt = p_qk.tile([128, T], f16, name=f"kt{b}", tag="qk")
                for w_sb, x_sb, b_sb, dst in (
                    (wk_sb, xk, bk_sb, kt),
                    (wq_sb, xq, bq_sb, qt),
                ):
                    for j in range(NTQB):
                        ps = ps_sc.tile([128, TQB], f32, name=f"psqk{b}{j}", tag="sc")
                        for dc in range(NDC):
                            nc.tensor.matmul(
                                ps[:],
                                lhsT=w_sb[:, dc * NW : (dc + 1) * NW],
                                rhs=x_sb[
                                    :, dc * T + j * TQB : dc * T + (j + 1) * TQB
                                ],
                                start=(dc == 0),
                                stop=(dc == NDC - 1),
                            )
                        nc.vector.tensor_scalar(
                            dst[:, j * TQB : (j + 1) * TQB],
                            ps[:],
                            b_sb[:, 0:1],
                            None,
                            ALU.add,
                        )
                return qt, kt, va

        # ===== attention: projections(b) then units(b), b=1 proj overlaps ====
        if True:
            for b in range(B):
                qt, kt, va = projections(b)
                for j in range(NTQB):
                    pv = [
                        ps_pv.tile([VA, TQB], f32, name=f"pv{b}{j}{hd}", tag="pv")
                        for hd in range(NLOC)
                    ]
                    pv_emitted = [0, 0]
                    # dual-rounds: tiles r and r+1 (i = r), PVs emitted in
                    # reversed tile order so the second pair needs no new wait
                    # (covered by the ACT-queue wait of the first pair).
                    for r0 in range(0, NTKC, 2):
                        pts = []
                        for r in (r0, r0 + 1):
                            pss = ps_sc.tile(
                                [128, SLOTS * TQB],
                                f32,
                                name=f"pss{b}{j}{r}",
                                tag="sc",
                            )
                            for hd in range(NLOC):
                                nc.tensor.matmul(
                                    pss[:, hd * TQB : (hd + 1) * TQB],
                                    lhsT=kt[
                                        hd * H : (hd + 1) * H,
                                        r * TKC : (r + 1) * TKC,
                                    ],
                                    rhs=qt[
                                        hd * H : (hd + 1) * H,
                                        j * TQB : (j + 1) * TQB,
                                    ],
                                    start=True,
                                    stop=True,
                                )
                            # exp evacuation: ACT reading PSUM throttles
                            # concurrent PE matmuls ~1.8x, DVE PSUM reads do
                            # not — but the DVE fp32->f16 CAST is 1x-slow, so
                            # alternate the two paths.
                            pt = p_pt.tile(
                                [128, SLOTS * TQB],
                                f16,
                                name=f"pt{b}{j}{r}",
                                tag="pt",
                            )
                            if r % 2 == 0:
                                nc.scalar.activation(pt[:], pss[:], AF.Exp)
                            else:
                                s_sb = p_pt.tile(
                                    [128, SLOTS * TQB],
                                    f16,
                                    name=f"ss{b}{j}{r}",
                                    tag="ss",
                                )
                                nc.vector.tensor_copy(s_sb[:], pss[:])
                                nc.scalar.activation(pt[:], s_sb[:], AF.Exp)
                            pts.append((r, pt))
                        for r, pt in reversed(pts):
                            for hd in range(NLOC):
                                col0 = r * NLOC * VA + hd * VA
                                nc.tensor.matmul(
                                    pv[hd][:],
                                    lhsT=va[:, col0 : col0 + VA],
                                    rhs=pt[:, hd * TQB : (hd + 1) * TQB],
                                    start=(pv_emitted[hd] == 0),
                                    stop=(pv_emitted[hd] == NTKC - 1),
                                )
                                pv_emitted[hd] += 1
                    # normalize + stage shard (b*4 + j) for AllToAll. The
                    # reciprocal row is replicated across 64 partitions by
                    # bouncing through DRAM (DMA reads may repeat a DRAM
                    # source; SBUF APs cannot have zero partition step).
                    for hd in range(NLOC):
                        a_sb = p_a.tile(
                            [H + 1, TQB], f32, name=f"a{b}{j}{hd}", tag="a"
                        )
                        nc.vector.tensor_copy(a_sb[:], pv[hd][0 : H + 1, :])
                        dscr = p_dram.tile(
                            [1, TQB], f32, name=f"ds{b}{j}{hd}", tag="ds", bufs=3
                        )
                        nc.sync.dma_start(dscr[:], a_sb[H : H + 1, :])
                        dn = p_a.tile(
                            [128, TQB // 128], f32, name=f"dn{b}{j}{hd}", tag="dn"
                        )
                        nc.sync.dma_start(
                            dn[:], dscr[:].rearrange("o (p x) -> (o p) x", p=128)
                        )
                        rc = p_a.tile(
                            [128, TQB // 128], f32, name=f"rc{b}{j}{hd}", tag="rc"
                        )
                        nc.vector.reciprocal(rc[:], dn[:])
                        rscr = p_dram.tile(
                            [1, TQB], f32, name=f"rs{b}{j}{hd}", tag="rs", bufs=3
                        )
                        nc.sync.dma_start(
                            rscr[:].rearrange("o (p x) -> (o p) x", p=128), rc[:]
                        )
                        rep = p_a.tile([H, TQB], f32, name=f"rp{b}{j}{hd}", tag="rep")
                        nc.sync.dma_start(
                            rep[:], rscr[0:1, :].to_broadcast((H, TQB))
                        )
                        an = p_a.tile([H, TQB], f16, name=f"an{b}{j}{hd}", tag="an")
                        nc.vector.tensor_tensor(an[:], a_sb[0:H, :], rep[:], ALU.mult)
                        r0_ = (b * NTQB + j) * NW + hd * H
                        nc.sync.dma_start(a2a_in[r0_ : r0_ + H, :], an[:])

        # ---- AllToAll: head shards -> (batch, seq-block) shards -------------
        nc.gpsimd.collective_compute(
            "AllToAll",
            ALU.bypass,
            replica_groups=GROUPS,
            ins=[a2a_in.opt()],
            outs=[a2a_out.opt()],
        )

        # ---- output projection for this core's 512-row slice ----------------

        NHC = N_HEADS * H // 128  # 8 nh chunks
        ats = []
        for nhc in range(NHC):
            at = p_at.tile([128, TQB], f16, name=f"at{nhc}", tag="at")
            nc.sync.dma_start(at[:], a2a_out[nhc * 128 : (nhc + 1) * 128, :])
            ats.append(at)
        for tqc in range(OUT_ROWS // 128):  # 4
            for dh in range(2):
                pso = ps_sc.tile([128, 512], f32, name=f"pso{tqc}{dh}", tag="sc")
                for nhc in range(NHC):
                    nc.tensor.matmul(
                        pso[:],
                        lhsT=ats[nhc][:, tqc * 128 : (tqc + 1) * 128],
                        rhs=wp_sb[:, nhc * D + dh * 512 : nhc * D + (dh + 1) * 512],
                        start=(nhc == 0),
                        stop=(nhc == NHC - 1),
                    )
                o_sb = p_o.tile([128, 512], f32, name=f"o{tqc}{dh}", tag="o")
                nc.vector.tensor_tensor(
                    o_sb[:],
                    pso[:],
                    bp_sb[:, dh * 512 : (dh + 1) * 512],
                    ALU.add,
                )
                nc.sync.dma_start(
                    out[tqc * 128 : (tqc + 1) * 128, dh * 512 : (dh + 1) * 512],
                    o_sb[:],
                )

    orig_to_json = nc.to_json_bytes
    nc.to_json_bytes = lambda: _legalize_waits(orig_to_json())
    return nc


def _get_nc():
    if "nc" not in _CACHE:
        _CACHE["nc"] = _build()
    return _CACHE["nc"]


def _make_in_maps(inputs):
    q = np.asarray(inputs["q"], dtype=np.float32)
    v = np.asarray(inputs["v"], dtype=np.float32)
    k = np.asarray(inputs["k"], dtype=np.float32)
    w_query = np.asarray(inputs["w_query"], dtype=np.float32)
    b_query = np.asarray(inputs["b_query"], dtype=np.float32)
    w_value = np.asarray(inputs["w_value"], dtype=np.float32)
    b_value = np.asarray(inputs["b_value"], dtype=np.float32)
    w_key = np.asarray(inputs["w_key"], dtype=np.float32)
    b_key = np.asarray(inputs["b_key"], dtype=np.float32)
    w_projection = np.asarray(inputs["w_projection"], dtype=np.float32)
    b_projection = np.asarray(inputs["b_projection"], dtype=np.float32)

    scale = np.float32(1.0 / np.sqrt(H))
    wp_s = np.ascontiguousarray(
        w_projection.transpose(0, 2, 1).reshape(N_HEADS * H, D)
    ).astype(F16)
    bp_s = np.ascontiguousarray(
        np.tile(b_projection.reshape(1, D), (128, 1))
    ).astype(np.float32)

    xT = {}
    for b in range(B):
        xT[b] = tuple(
            np.ascontiguousarray(x[b].T).astype(F16) for x in (q, k, v)
        )

    in_maps = []
    for c in range(NCORES):
        hs = c * NLOC
        wq_s = (w_query[:, hs : hs + NLOC, :].reshape(D, NW) * scale).astype(F16)
        wk_s = w_key[:, hs : hs + NLOC, :].reshape(D, NW).astype(F16)
        wv_s = w_value[:, hs : hs + NLOC, :].reshape(D, NW).astype(F16)
        bq_s = np.ascontiguousarray(
            (b_query[hs : hs + NLOC].reshape(NW) * scale).reshape(NW, 1)
        ).astype(np.float32)
        bk_s = np.ascontiguousarray(
            b_key[hs : hs + NLOC].reshape(NW, 1)
        ).astype(np.float32)
        bv_s = np.ascontiguousarray(
            b_value[hs : hs + NLOC].reshape(NW, 1)
        ).astype(np.float32)
        m = {
            "ident": np.eye(128, dtype=np.float32).astype(F16),
            "wq": np.ascontiguousarray(wq_s),
            "wk": np.ascontiguousarray(wk_s),
            "wv": np.ascontiguousarray(wv_s),
            "wp": wp_s,
            "bq": bq_s,
            "bk": bk_s,
            "bv": bv_s,
            "bp": bp_s,
        }
        for b in range(B):
            m[f"qT{b}"], m[f"kT{b}"], m[f"vT{b}"] = xT[b]
        in_maps.append(m)
    return in_maps


def _assemble(results):
    out = np.empty((B, T, D), np.float32)
    for c in range(NCORES):
        b = c // 4
        j = c % 4
        out[b, j * OUT_ROWS : (j + 1) * OUT_ROWS, :] = results[c]["out"]
    return out


def run(inputs, trace=False, **kwargs):
    from concourse.bass_utils import run_bass_kernel_spmd

    nc = _get_nc()
    in_maps = _make_in_maps(inputs)
    res = run_bass_kernel_spmd(
        nc, in_maps, list(range(NCORES)), trace=trace, **kwargs
    )
    return _assemble(res.results), res


def kernel(**inputs) -> np.ndarray:
    out, _ = run(inputs, trace=False)
    return out

